# revision 14
# baseline (speedup 1.0000x reference)
"""AlphaRankingLoss on 8 TRN2 NeuronCores (Bass SPMD kernel).

reference math:
    mse  = mean((p - t)^2)
    pair_loss[i,j] = relu(0.5 - sign(t_i - t_j) * (p_i - p_j)),  i<j, t_i != t_j
    loss = 0.7 * mse + 0.3 * sum(pair_loss)/count

The pair term is symmetric under (i,j) swap, so the triu sum over non-tied
pairs equals the sum over ordered pairs with t_i > t_j of
relu(0.5 - (p_i - p_j)); the count equals #{(i,j): t_i > t_j}.  Each core
computes 1024 rows x 8192 cols of that ordered-pair matrix, tiled as
[128 rows x CHUNK cols]:

  VectorE: pen[i,j] = -BIG * (t_j >= t_i)        tensor_scalar is_ge+mult (4x)
           arg[i,j] = pen[i,j] + p_j             tensor_tensor add (2x)
  ScalarE: y = relu(arg + (0.5 - p_i)), fused row-sum accumulate
           (a subset of tiles does this on VectorE instead, with the sums
            taken by TensorE, to balance the two engines)
  TensorE: ones^T @ pen tiles accumulated into PSUM -> -BIG * pair count
           ones^T @ y tiles (VectorE-relu'd subset) -> their pair sums

The per-core partials (pair sum, -BIG*count, mse sum) are AllReduce'd
across the 8 cores and the final scalar math runs on VectorE.
"""

import numpy as np

import concourse.bass as bass
import concourse.mybir as mybir
from concourse.bass_utils import run_bass_kernel_spmd

# ---- problem constants (hardcoded per spec) ----
B = 8192
NCORES = 8
ROWS_PER_CORE = B // NCORES  # 1024
P = 128
RT = ROWS_PER_CORE // P  # 8 row tiles per core
ALPHA = 0.7
MARGIN = 0.5
BIG = float(2**30)

F32 = mybir.dt.float32
BF16 = mybir.dt.bfloat16

MODE = "bf16"


def build(mode=None):
    mode = mode or MODE
    if mode == "f32":
        cdt = F32
        adt = F32
        chunk = 4096
        vrelu_every = 0  # all relu on ScalarE
    else:
        cdt = BF16
        adt = BF16
        chunk = 4096
        vrelu_every = 0  # all relu on ScalarE
    nch = B // chunk
    nt = nch * RT
    csz = chunk
    DEPTH = 4

    VSET = [i for i in range(nt) if vrelu_every and i % vrelu_every == vrelu_every - 1]
    SSET = [i for i in range(nt) if i not in VSET]
    ns, nv = len(SSET), len(VSET)
    s_idx = {i: si for si, i in enumerate(SSET)}
    v_idx = {i: vi for vi, i in enumerate(VSET)}

    nc = bass.Bass(num_devices=NCORES, detect_race_conditions=False)

    t_rows = nc.declare_dram_parameter("t_rows", [P, RT], F32, isOutput=False)
    p_rows = nc.declare_dram_parameter("p_rows", [P, RT], F32, isOutput=False)
    t_full = nc.declare_dram_parameter("t_full", [1, B], F32, isOutput=False)
    p_full = nc.declare_dram_parameter("p_full", [1, B], F32, isOutput=False)
    out_ext = nc.declare_dram_parameter("out", [1, 1], F32, isOutput=True)

    tbf_dram = nc.dram_tensor("tbf_dram", [1, B], cdt)
    pbf_dram = nc.dram_tensor("pbf_dram", [1, B], adt)
    warm_out = nc.dram_tensor("warm_out", [1, 128], cdt, addr_space="Shared")
    cc_in = nc.dram_tensor("cc_in", [1, 128], F32)
    cc_out = nc.dram_tensor("cc_out", [1, 128], F32, addr_space="Shared")

    core_ids = list(range(NCORES))
    AX = mybir.AxisListType.X
    OP = mybir.AluOpType
    AF = mybir.ActivationFunctionType

    from contextlib import ExitStack

    ctx = ExitStack()
    with ctx:
        tb = [ctx.enter_context(nc.sbuf_tensor(f"tb{b}", [P, csz], cdt)) for b in range(2)]
        pb = [ctx.enter_context(nc.sbuf_tensor(f"pb{b}", [P, csz], adt)) for b in range(2)]
        pen = [ctx.enter_context(nc.sbuf_tensor(f"pen{b}", [P, csz], adt)) for b in range(DEPTH)]
        argb = [ctx.enter_context(nc.sbuf_tensor(f"argb{b}", [P, csz], adt)) for b in range(DEPTH)]
        yscr = [ctx.enter_context(nc.sbuf_tensor(f"yscr{b}", [P, csz], adt)) for b in range(2)]
        yv = [ctx.enter_context(nc.sbuf_tensor(f"yv{b}", [P, csz], adt)) for b in range(2)]
        t_sb = ctx.enter_context(nc.sbuf_tensor("t_sb", [P, RT], F32))
        p_sb = ctx.enter_context(nc.sbuf_tensor("p_sb", [P, RT], F32))
        tqh_sb = ctx.enter_context(nc.sbuf_tensor("tqh_sb", [P, RT], cdt))
        tq_sb = ctx.enter_context(nc.sbuf_tensor("tq_sb", [P, RT], F32))
        bias_sb = ctx.enter_context(nc.sbuf_tensor("bias_sb", [P, RT], F32))
        d_sb = ctx.enter_context(nc.sbuf_tensor("d_sb", [P, RT], F32))
        d2_sb = ctx.enter_context(nc.sbuf_tensor("d2_sb", [P, RT], F32))
        # red_src columns: [0:ns] ScalarE y sums, [ns] mse
        red_src = ctx.enter_context(nc.sbuf_tensor("red_src", [P, ns + 1], F32))
        ones_sb = ctx.enter_context(nc.sbuf_tensor("ones_sb", [P, 1], F32))
        ones_bf = ctx.enter_context(nc.sbuf_tensor("ones_bf", [P, 1], adt))
        cc_sb = ctx.enter_context(nc.sbuf_tensor("cc_sb", [1, 128], F32))
        gsb = ctx.enter_context(nc.sbuf_tensor("gsb", [1, 128], F32))
        tmp = ctx.enter_context(nc.sbuf_tensor("tmp", [1, 8], F32))
        out_sb = ctx.enter_context(nc.sbuf_tensor("out_sb", [1, 1], F32))
        psum_h = ctx.enter_context(nc.psum_tensor("psum_h", [1, 512], F32))
        psum_red = ctx.enter_context(nc.psum_tensor("psum_red", [1, ns + 1], F32))

        sem_ld = ctx.enter_context(nc.semaphore("sem_ld"))
        sem_pc = ctx.enter_context(nc.semaphore("sem_pc"))
        sem_bct = [ctx.enter_context(nc.semaphore(f"sem_bct{b}")) for b in range(2)]
        sem_bcp = [ctx.enter_context(nc.semaphore(f"sem_bcp{b}")) for b in range(2)]
        sem_vchunk = ctx.enter_context(nc.semaphore("sem_vchunk"))
        sem_pen = ctx.enter_context(nc.semaphore("sem_pen"))
        sem_tt = ctx.enter_context(nc.semaphore("sem_tt"))
        sem_arg = ctx.enter_context(nc.semaphore("sem_arg"))
        sem_y = ctx.enter_context(nc.semaphore("sem_y"))
        sem_yv = ctx.enter_context(nc.semaphore("sem_yv"))
        sem_ty = ctx.enter_context(nc.semaphore("sem_ty"))
        sem_vdone = ctx.enter_context(nc.semaphore("sem_vdone"))
        sem_t = ctx.enter_context(nc.semaphore("sem_t"))
        sem_vcc = ctx.enter_context(nc.semaphore("sem_vcc"))
        sem_vout = ctx.enter_context(nc.semaphore("sem_vout"))
        sem_dcc = ctx.enter_context(nc.semaphore("sem_dcc"))
        cc_sem = ctx.enter_context(nc.semaphore("cc_sem"))

        block = ctx.enter_context(nc.Block())

        cast_bcast = cdt != F32 or adt != F32

        @block.sync
        def _(sync: bass.BassEngine):
            sync.dma_start(out=t_sb[:, :], in_=t_rows[:, :]).then_inc(sem_ld, 16)
            sync.dma_start(out=p_sb[:, :], in_=p_rows[:, :]).then_inc(sem_ld, 16)
            # t-broadcasts from the bf16 scratch (gpsimd does p-broadcasts)
            sync.wait_ge(sem_pc, 32)
            for k in range(nch):
                bsem = sem_bct[k % 2]
                if k >= 2:
                    sync.wait_ge(sem_vchunk, k - 1)
                    sync.wait_ge(bsem, 16 * (k // 2))
                src_t = tbf_dram[0:1, k * csz : (k + 1) * csz].partition_broadcast(P)
                sync.dma_start(out=tb[k % 2][:, :], in_=src_t).then_inc(bsem, 16)
            # collective staging
            sync.wait_ge(sem_vcc, 1)
            sync.dma_start(out=cc_in[0:1, :], in_=cc_sb[0:1, :]).then_inc(sem_dcc, 16)
            sync.wait_ge(cc_sem, 2)
            sync.dma_start(out=gsb[0:1, :], in_=cc_out[0:1, :]).then_inc(sem_dcc, 16)
            sync.wait_ge(sem_vout, 1)
            sync.dma_start(out=out_ext[0:1, :], in_=out_sb[0:1, :]).then_inc(sem_dcc, 16)
            sync.wait_ge(sem_dcc, 48)

        @block.gpsimd
        def _(gpsimd: bass.BassGpSimd):
            # pre-cast f32 -> compute dtype into DRAM scratch (SWDGE casts)
            gpsimd.dma_start(out=tbf_dram[0:1, :], in_=t_full[0:1, :]).then_inc(sem_pc, 16)
            gpsimd.dma_start(out=pbf_dram[0:1, :], in_=p_full[0:1, :]).then_inc(sem_pc, 16)
            # warm up the collective firmware; result unused, overlaps compute
            gpsimd.wait_ge(sem_pc, 16)
            gpsimd.collective_compute(
                "AllReduce",
                OP.add,
                replica_groups=[core_ids],
                ins=[tbf_dram[0:1, 0:128]],
                outs=[warm_out[0:1, :]],
            ).then_inc(cc_sem)
            gpsimd.wait_ge(sem_pc, 32)
            for k in range(nch):
                bsem = sem_bcp[k % 2]
                if k >= 2:
                    gpsimd.wait_ge(sem_vchunk, k - 1)
                    gpsimd.wait_ge(bsem, 16 * (k // 2))
                src_p = pbf_dram[0:1, k * csz : (k + 1) * csz].partition_broadcast(P)
                gpsimd.dma_start(out=pb[k % 2][:, :], in_=src_p).then_inc(bsem, 16)
            gpsimd.wait_ge(sem_dcc, 16)
            gpsimd.collective_compute(
                "AllReduce",
                OP.add,
                replica_groups=[core_ids],
                ins=[cc_in[0:1, :]],
                outs=[cc_out[0:1, :]],
            ).then_inc(cc_sem)

        @block.vector
        def _(vector: bass.BassVectorEngine):
            vector.memset(ones_sb[:, :], 1.0)
            vector.memset(ones_bf[:, :], 1.0)
            vector.memset(cc_sb[0:1, :], 0.0)
            vector.wait_ge(sem_ld, 32)
            # bias = MARGIN - p
            vector.tensor_scalar(
                out=bias_sb[:, :], in0=p_sb[:, :],
                scalar1=-1.0, scalar2=MARGIN, op0=OP.mult, op1=OP.add,
            )
            # t rounded through the compare dtype, back to f32, so the per-row
            # scalar and the broadcast tensor quantize identically
            vector.tensor_copy(out=tqh_sb[:, :], in_=t_sb[:, :])
            # mse partial: sum((p - t)^2) over this core's rows
            vector.tensor_tensor(
                out=d_sb[:, :], in0=p_sb[:, :], in1=t_sb[:, :], op=OP.subtract
            )
            vector.drain()
            vector.tensor_copy(out=tq_sb[:, :], in_=tqh_sb[:, :])
            vector.scalar_tensor_tensor(
                out=d2_sb[:, :], in0=d_sb[:, :], scalar=0.0, in1=d_sb[:, :],
                op0=OP.add, op1=OP.mult,
                accum_out=red_src[:, ns : ns + 1],
            )
            vector.drain()
            i = 0
            for k in range(nch):
                vector.wait_ge(sem_bct[k % 2], 16 * (k // 2 + 1))
                vector.wait_ge(sem_bcp[k % 2], 16 * (k // 2 + 1))
                for r in range(RT):
                    # pen = -BIG * (t_j >= t_i)
                    if i >= DEPTH:
                        vector.wait_ge(sem_tt, i - DEPTH + 1)
                    vector.tensor_scalar(
                        out=pen[i % DEPTH][:, :], in0=tb[k % 2][:, :],
                        scalar1=tq_sb[:, r : r + 1], scalar2=-BIG,
                        op0=OP.is_ge, op1=OP.mult,
                    ).then_inc(sem_pen, 1)
                    # arg = pen + p_j
                    if i >= DEPTH and (i - DEPTH) in s_idx:
                        vector.wait_ge(sem_y, s_idx[i - DEPTH] + 1)
                    vector.tensor_tensor(
                        out=argb[i % DEPTH][:, :], in0=pen[i % DEPTH][:, :],
                        in1=pb[k % 2][:, :], op=OP.add,
                    ).then_inc(sem_arg, 1)
                    if i in v_idx:
                        vi = v_idx[i]
                        if vi >= 2:
                            vector.wait_ge(sem_ty, vi - 1)
                        # y = relu(arg + bias)
                        vector.tensor_scalar(
                            out=yv[vi % 2][:, :], in0=argb[i % DEPTH][:, :],
                            scalar1=bias_sb[:, r : r + 1], scalar2=0.0,
                            op0=OP.add, op1=OP.max,
                        ).then_inc(sem_yv, 1)
                    i += 1
                vector.memset(tmp[0:1, 7:8], 0.0).then_inc(sem_vchunk, 1)
            vector.memset(tmp[0:1, 6:7], 0.0).then_inc(sem_vdone, 1)

            # ---- final reduction ----
            vector.wait_ge(sem_t, 1)
            vector.wait_ge(sem_tt, nt)
            if nv:
                vector.wait_ge(sem_ty, nv)
            # pair sum = sum(ScalarE col sums) + sum(psum_y)
            vector.tensor_reduce(
                out=tmp[0:1, 0:1], in_=psum_red[0:1, 0:ns], axis=AX, op=OP.add
            )
            vector.memset(tmp[0:1, 1:2], 0.0)
            vector.tensor_reduce(
                out=cc_sb[0:1, 1:2], in_=psum_h[0:1, :], axis=AX, op=OP.add
            )
            vector.tensor_copy(out=cc_sb[0:1, 2:3], in_=psum_red[0:1, ns : ns + 1])
            vector.drain()
            vector.tensor_tensor(
                out=cc_sb[0:1, 0:1], in0=tmp[0:1, 0:1], in1=tmp[0:1, 1:2], op=OP.add
            )
            vector.drain()
            vector.memset(tmp[0:1, 5:6], 0.0).then_inc(sem_vcc, 1)

            vector.wait_ge(sem_dcc, 32)
            # gsb[0,0]=S, gsb[0,1]=-BIG*count (+noise), gsb[0,2]=mse sum
            # C = B^2 + gsb[0,1]/BIG
            vector.tensor_scalar(
                out=tmp[0:1, 0:1], in0=gsb[0:1, 1:2],
                scalar1=1.0 / BIG, scalar2=float(B * B), op0=OP.mult, op1=OP.add,
            )
            vector.drain()
            vector.tensor_scalar(
                out=tmp[0:1, 1:2], in0=tmp[0:1, 0:1], scalar1=1.0, scalar2=None,
                op0=OP.max,
            )
            vector.drain()
            vector.reciprocal(out=tmp[0:1, 2:3], in_=tmp[0:1, 1:2])
            vector.drain()
            vector.tensor_tensor(
                out=tmp[0:1, 3:4], in0=gsb[0:1, 0:1], in1=tmp[0:1, 2:3], op=OP.mult
            )
            vector.tensor_scalar(
                out=tmp[0:1, 4:5], in0=tmp[0:1, 0:1], scalar1=0.5, scalar2=None,
                op0=OP.is_ge,
            )
            vector.drain()
            vector.tensor_tensor(
                out=tmp[0:1, 5:6], in0=tmp[0:1, 3:4], in1=tmp[0:1, 4:5], op=OP.mult
            )
            vector.tensor_scalar(
                out=tmp[0:1, 6:7], in0=gsb[0:1, 2:3], scalar1=ALPHA / B, scalar2=None,
                op0=OP.mult,
            )
            vector.drain()
            vector.scalar_tensor_tensor(
                out=out_sb[0:1, 0:1], in0=tmp[0:1, 5:6], scalar=1.0 - ALPHA,
                in1=tmp[0:1, 6:7], op0=OP.mult, op1=OP.add,
            ).then_inc(sem_vout, 1)

        @block.scalar
        def _(scalar: bass.BassScalarEngine):
            for i in SSET:
                r = i % RT
                si = s_idx[i]
                scalar.wait_ge(sem_arg, i + 1)
                scalar.activation(
                    out=yscr[si % 2][:, :], in_=argb[i % DEPTH][:, :], func=AF.Relu,
                    bias=bias_sb[:, r : r + 1], scale=1.0,
                    accum_out=red_src[:, si : si + 1],
                ).then_inc(sem_y, 1)

        @block.tensor
        def _(tensor: bass.BassTensorEngine):
            nslice = csz // 512
            for i in range(nt):
                tensor.wait_ge(sem_pen, i + 1)
                for s in range(nslice):
                    mm = tensor.matmul(
                        out=psum_h[0:1, :],
                        lhsT=ones_bf[:, 0:1],
                        rhs=pen[i % DEPTH][:, s * 512 : (s + 1) * 512],
                        start=(i == 0 and s == 0),
                        stop=(i == nt - 1 and s == nslice - 1),
                    )
                    if s == nslice - 1:
                        mm.then_inc(sem_tt, 1)
            # final reduction of ScalarE sums + mse
            tensor.wait_ge(sem_vdone, 1)
            tensor.wait_ge(sem_y, ns)
            tensor.matmul(
                out=psum_red[0:1, :],
                lhsT=ones_sb[:, 0:1],
                rhs=red_src[:, :],
                start=True,
                stop=True,
            ).then_inc(sem_t, 1)

    return nc


_NC_CACHE = {}


def _get_nc(mode=None):
    mode = mode or MODE
    if mode not in _NC_CACHE:
        _NC_CACHE[mode] = build(mode)
    return _NC_CACHE[mode]


def make_in_maps(predictions: np.ndarray, targets: np.ndarray):
    p = np.ascontiguousarray(predictions.reshape(B).astype(np.float32))
    t = np.ascontiguousarray(targets.reshape(B).astype(np.float32))
    in_maps = []
    for c in range(NCORES):
        sl = slice(c * ROWS_PER_CORE, (c + 1) * ROWS_PER_CORE)
        in_maps.append(
            {
                "t_rows": t[sl].reshape(P, RT).copy(),
                "p_rows": p[sl].reshape(P, RT).copy(),
                "t_full": t.reshape(1, B).copy(),
                "p_full": p.reshape(1, B).copy(),
            }
        )
    return in_maps


def kernel(predictions: np.ndarray, targets: np.ndarray) -> np.ndarray:
    nc = _get_nc()
    in_maps = make_in_maps(predictions, targets)
    res = run_bass_kernel_spmd(nc, in_maps, core_ids=list(range(NCORES)))
    out = res.results[0]["out"]
    return np.float32(out.reshape(())[()])


# revision 20
# speedup vs baseline: 1.0418x; 1.0418x over previous
"""AlphaRankingLoss on 8 TRN2 NeuronCores (Bass SPMD kernel).

reference math:
    mse  = mean((p - t)^2)
    pair_loss[i,j] = relu(0.5 - sign(t_i - t_j) * (p_i - p_j)),  i<j, t_i != t_j
    loss = 0.7 * mse + 0.3 * sum(pair_loss)/count

The pair term is symmetric under (i,j) swap, so the triu sum over non-tied
pairs equals the sum over ordered pairs with t_i > t_j of
relu(0.5 - (p_i - p_j)); the count equals #{(i,j): t_i > t_j}.  Each core
computes 1024 rows x 8192 cols of that ordered-pair matrix, tiled as
[128 rows x CHUNK cols]:

  VectorE: pen[i,j] = -BIG * (t_j >= t_i)        tensor_scalar is_ge+mult (4x)
           arg[i,j] = pen[i,j] + p_j             tensor_tensor add (2x)
  ScalarE: y = relu(arg + (0.5 - p_i)), fused row-sum accumulate
           (a subset of tiles does this on VectorE instead, with the sums
            taken by TensorE, to balance the two engines)
  TensorE: ones^T @ pen tiles accumulated into PSUM -> -BIG * pair count
           ones^T @ y tiles (VectorE-relu'd subset) -> their pair sums

The per-core partials (pair sum, -BIG*count, mse sum) are AllReduce'd
across the 8 cores and the final scalar math runs on VectorE.
"""

import numpy as np

import concourse.bass as bass
import concourse.mybir as mybir
from concourse.bass_utils import run_bass_kernel_spmd

# ---- problem constants (hardcoded per spec) ----
B = 8192
NCORES = 8
ROWS_PER_CORE = B // NCORES  # 1024
P = 128
RT = ROWS_PER_CORE // P  # 8 row tiles per core
ALPHA = 0.7
MARGIN = 0.5
BIG = float(2**30)

F32 = mybir.dt.float32
BF16 = mybir.dt.bfloat16

MODE = "bf16"


def build(mode=None):
    mode = mode or MODE
    if mode == "f32":
        cdt = F32
        adt = F32
        chunk = 4096
        vrelu_every = 0  # all relu on ScalarE
    else:
        cdt = BF16
        adt = BF16
        chunk = 4096
        vrelu_every = 1  # last tiles relu on VectorE
    nch = B // chunk
    nt = nch * RT
    csz = chunk
    DEPTH = 4

    if vrelu_every:
        VSET = [nt - 2, nt - 1]
    else:
        VSET = []
    SSET = [i for i in range(nt) if i not in VSET]
    ns, nv = len(SSET), len(VSET)
    s_idx = {i: si for si, i in enumerate(SSET)}
    v_idx = {i: vi for vi, i in enumerate(VSET)}

    nc = bass.Bass(num_devices=NCORES, detect_race_conditions=False)

    t_rows = nc.declare_dram_parameter("t_rows", [P, RT], F32, isOutput=False)
    p_rows = nc.declare_dram_parameter("p_rows", [P, RT], F32, isOutput=False)
    t_full = nc.declare_dram_parameter("t_full", [1, B], F32, isOutput=False)
    p_full = nc.declare_dram_parameter("p_full", [1, B], F32, isOutput=False)
    out_ext = nc.declare_dram_parameter("out", [1, 1], F32, isOutput=True)

    cc_in = nc.dram_tensor("cc_in", [1, 128], F32)
    cc_out = nc.dram_tensor("cc_out", [1, 128], F32, addr_space="Shared")
    tbf_dram = nc.dram_tensor("tbf_dram", [1, B], cdt)
    pbf_dram = nc.dram_tensor("pbf_dram", [1, B], adt)

    core_ids = list(range(NCORES))
    AX = mybir.AxisListType.X
    OP = mybir.AluOpType
    AF = mybir.ActivationFunctionType

    from contextlib import ExitStack

    ctx = ExitStack()
    with ctx:
        tb = [ctx.enter_context(nc.sbuf_tensor(f"tb{b}", [P, csz], cdt)) for b in range(2)]
        pb = [ctx.enter_context(nc.sbuf_tensor(f"pb{b}", [P, csz], adt)) for b in range(2)]
        pen = [ctx.enter_context(nc.sbuf_tensor(f"pen{b}", [P, csz], adt)) for b in range(DEPTH)]
        argb = [ctx.enter_context(nc.sbuf_tensor(f"argb{b}", [P, csz], adt)) for b in range(DEPTH)]
        yscr = [ctx.enter_context(nc.sbuf_tensor(f"yscr{b}", [P, csz], adt)) for b in range(2)]
        yv = [ctx.enter_context(nc.sbuf_tensor(f"yv{b}", [P, csz], adt)) for b in range(2)]
        t_sb = ctx.enter_context(nc.sbuf_tensor("t_sb", [P, RT], F32))
        p_sb = ctx.enter_context(nc.sbuf_tensor("p_sb", [P, RT], F32))
        tqh_sb = ctx.enter_context(nc.sbuf_tensor("tqh_sb", [P, RT], cdt))
        tq_sb = ctx.enter_context(nc.sbuf_tensor("tq_sb", [P, RT], F32))
        bias_sb = ctx.enter_context(nc.sbuf_tensor("bias_sb", [P, RT], F32))
        d_sb = ctx.enter_context(nc.sbuf_tensor("d_sb", [P, RT], F32))
        d2_sb = ctx.enter_context(nc.sbuf_tensor("d2_sb", [P, RT], F32))
        # red_src columns: [0:ns] ScalarE y sums, [ns] mse
        red_src = ctx.enter_context(nc.sbuf_tensor("red_src", [P, ns + 1], F32))
        ones_sb = ctx.enter_context(nc.sbuf_tensor("ones_sb", [P, 1], F32))
        ones_bf = ctx.enter_context(nc.sbuf_tensor("ones_bf", [P, 1], adt))
        send_sb = ctx.enter_context(nc.sbuf_tensor("send_sb", [1, 128], F32))
        gsb = ctx.enter_context(nc.sbuf_tensor("gsb", [1, 128], F32))
        tmp = ctx.enter_context(nc.sbuf_tensor("tmp", [1, 8], F32))
        out_sb = ctx.enter_context(nc.sbuf_tensor("out_sb", [1, 1], F32))
        psum_h = ctx.enter_context(nc.psum_tensor("psum_h", [1, 512], F32))
        psum_y = ctx.enter_context(nc.psum_tensor("psum_y", [1, 512], F32))
        psum_red = ctx.enter_context(nc.psum_tensor("psum_red", [1, ns + 1], F32))

        sem_ld = ctx.enter_context(nc.semaphore("sem_ld"))
        sem_pc = ctx.enter_context(nc.semaphore("sem_pc"))
        sem_bct = [ctx.enter_context(nc.semaphore(f"sem_bct{b}")) for b in range(2)]
        sem_bcp = [ctx.enter_context(nc.semaphore(f"sem_bcp{b}")) for b in range(2)]
        sem_vchunk = ctx.enter_context(nc.semaphore("sem_vchunk"))
        sem_pen = ctx.enter_context(nc.semaphore("sem_pen"))
        sem_tt = ctx.enter_context(nc.semaphore("sem_tt"))
        sem_arg = ctx.enter_context(nc.semaphore("sem_arg"))
        sem_y = ctx.enter_context(nc.semaphore("sem_y"))
        sem_yv = ctx.enter_context(nc.semaphore("sem_yv"))
        sem_ty = ctx.enter_context(nc.semaphore("sem_ty"))
        sem_vdone = ctx.enter_context(nc.semaphore("sem_vdone"))
        sem_t = ctx.enter_context(nc.semaphore("sem_t"))
        sem_vcc = ctx.enter_context(nc.semaphore("sem_vcc"))
        sem_vout = ctx.enter_context(nc.semaphore("sem_vout"))
        sem_dcc = ctx.enter_context(nc.semaphore("sem_dcc"))
        cc_sem = ctx.enter_context(nc.semaphore("cc_sem"))

        block = ctx.enter_context(nc.Block())

        cast_bcast = cdt != F32 or adt != F32

        @block.sync
        def _(sync: bass.BassEngine):
            sync.dma_start(out=t_sb[:, :], in_=t_rows[:, :]).then_inc(sem_ld, 16)
            sync.dma_start(out=p_sb[:, :], in_=p_rows[:, :]).then_inc(sem_ld, 16)
            # t-broadcasts from the bf16 scratch (gpsimd does p-broadcasts)
            sync.wait_ge(sem_pc, 32)
            for k in range(nch):
                bsem = sem_bct[k % 2]
                if k >= 2:
                    sync.wait_ge(sem_vchunk, k - 1)
                    sync.wait_ge(bsem, 16 * (k // 2))
                src_t = tbf_dram[0:1, k * csz : (k + 1) * csz].partition_broadcast(P)
                sync.dma_start(out=tb[k % 2][:, :], in_=src_t).then_inc(bsem, 16)
            sync.wait_ge(sem_vcc, 1)
            sync.dma_start(out=cc_in[0:1, :], in_=send_sb[0:1, :]).then_inc(sem_dcc, 16)
            sync.wait_ge(cc_sem, 1)
            sync.dma_start(out=gsb[0:1, 0:4], in_=cc_out[0:1, 0:4]).then_inc(sem_dcc, 16)
            sync.wait_ge(sem_vout, 1)
            sync.dma_start(out=out_ext[0:1, :], in_=out_sb[0:1, :]).then_inc(sem_dcc, 16)
            sync.wait_ge(sem_dcc, 48)

        @block.gpsimd
        def _(gpsimd: bass.BassGpSimd):
            # pre-cast f32 -> compute dtype into DRAM scratch (SWDGE casts)
            gpsimd.dma_start(out=tbf_dram[0:1, :], in_=t_full[0:1, :]).then_inc(sem_pc, 16)
            gpsimd.dma_start(out=pbf_dram[0:1, :], in_=p_full[0:1, :]).then_inc(sem_pc, 16)
            gpsimd.wait_ge(sem_pc, 32)
            for k in range(nch):
                bsem = sem_bcp[k % 2]
                if k >= 2:
                    gpsimd.wait_ge(sem_vchunk, k - 1)
                    gpsimd.wait_ge(bsem, 16 * (k // 2))
                src_p = pbf_dram[0:1, k * csz : (k + 1) * csz].partition_broadcast(P)
                gpsimd.dma_start(out=pb[k % 2][:, :], in_=src_p).then_inc(bsem, 16)
            gpsimd.wait_ge(sem_dcc, 16)
            gpsimd.collective_compute(
                "AllReduce",
                OP.add,
                replica_groups=[core_ids],
                ins=[cc_in[0:1, :]],
                outs=[cc_out[0:1, :]],
            ).then_inc(cc_sem)

        @block.vector
        def _(vector: bass.BassVectorEngine):
            vector.memset(ones_sb[:, :], 1.0)
            vector.memset(ones_bf[:, :], 1.0)
            vector.memset(send_sb[0:1, :], 0.0)
            vector.wait_ge(sem_ld, 32)
            # bias = MARGIN - p
            vector.tensor_scalar(
                out=bias_sb[:, :], in0=p_sb[:, :],
                scalar1=-1.0, scalar2=MARGIN, op0=OP.mult, op1=OP.add,
            )
            # t rounded through the compare dtype, back to f32, so the per-row
            # scalar and the broadcast tensor quantize identically
            vector.tensor_copy(out=tqh_sb[:, :], in_=t_sb[:, :])
            # mse partial: sum((p - t)^2) over this core's rows
            vector.tensor_tensor(
                out=d_sb[:, :], in0=p_sb[:, :], in1=t_sb[:, :], op=OP.subtract
            )
            vector.drain()
            vector.tensor_copy(out=tq_sb[:, :], in_=tqh_sb[:, :])
            vector.scalar_tensor_tensor(
                out=d2_sb[:, :], in0=d_sb[:, :], scalar=0.0, in1=d_sb[:, :],
                op0=OP.add, op1=OP.mult,
                accum_out=red_src[:, ns : ns + 1],
            )
            vector.drain()
            i = 0
            for k in range(nch):
                vector.wait_ge(sem_bct[k % 2], 16 * (k // 2 + 1))
                vector.wait_ge(sem_bcp[k % 2], 16 * (k // 2 + 1))
                for r in range(RT):
                    # pen = -BIG * (t_j >= t_i)
                    if i >= DEPTH:
                        vector.wait_ge(sem_tt, i - DEPTH + 1)
                    vector.tensor_scalar(
                        out=pen[i % DEPTH][:, :], in0=tb[k % 2][:, :],
                        scalar1=tq_sb[:, r : r + 1], scalar2=-BIG,
                        op0=OP.is_ge, op1=OP.mult,
                    ).then_inc(sem_pen, 1)
                    # arg = pen + p_j
                    if i >= DEPTH and (i - DEPTH) in s_idx:
                        vector.wait_ge(sem_y, s_idx[i - DEPTH] + 1)
                    vector.tensor_tensor(
                        out=argb[i % DEPTH][:, :], in0=pen[i % DEPTH][:, :],
                        in1=pb[k % 2][:, :], op=OP.add,
                    ).then_inc(sem_arg, 1)
                    if i in v_idx:
                        vi = v_idx[i]
                        if vi >= 2:
                            vector.wait_ge(sem_ty, vi - 1)
                        # y = relu(arg + bias)
                        vector.tensor_scalar(
                            out=yv[vi % 2][:, :], in0=argb[i % DEPTH][:, :],
                            scalar1=bias_sb[:, r : r + 1], scalar2=0.0,
                            op0=OP.add, op1=OP.max,
                        ).then_inc(sem_yv, 1)
                    i += 1
                vector.memset(tmp[0:1, 7:8], 0.0).then_inc(sem_vchunk, 1)
            vector.memset(tmp[0:1, 6:7], 0.0).then_inc(sem_vdone, 1)

            # ---- final reduction ----
            vector.wait_ge(sem_t, 1)
            vector.wait_ge(sem_tt, nt)
            if nv:
                vector.wait_ge(sem_ty, nv)
            # pair sum = sum(ScalarE col sums) + sum(psum_y)
            vector.tensor_reduce(
                out=tmp[0:1, 0:1], in_=psum_red[0:1, 0:ns], axis=AX, op=OP.add
            )
            if nv:
                vector.tensor_reduce(
                    out=tmp[0:1, 1:2], in_=psum_y[0:1, :], axis=AX, op=OP.add
                )
            else:
                vector.memset(tmp[0:1, 1:2], 0.0)
            vector.tensor_reduce(
                out=send_sb[0:1, 1:2], in_=psum_h[0:1, :], axis=AX, op=OP.add
            )
            vector.tensor_copy(out=send_sb[0:1, 2:3], in_=psum_red[0:1, ns : ns + 1])
            vector.drain()
            vector.tensor_tensor(
                out=send_sb[0:1, 0:1], in0=tmp[0:1, 0:1], in1=tmp[0:1, 1:2], op=OP.add
            )
            vector.drain()
            vector.memset(tmp[0:1, 5:6], 0.0).then_inc(sem_vcc, 1)

            vector.wait_ge(sem_dcc, 32)
            # gsb[0,0]=S, gsb[0,1]=-BIG*count (+noise), gsb[0,2]=mse sum
            # C = B^2 + gsb[0,1]/BIG
            vector.tensor_scalar(
                out=tmp[0:1, 0:1], in0=gsb[0:1, 1:2],
                scalar1=1.0 / BIG, scalar2=float(B * B), op0=OP.mult, op1=OP.add,
            )
            vector.drain()
            vector.tensor_scalar(
                out=tmp[0:1, 1:2], in0=tmp[0:1, 0:1], scalar1=1.0, scalar2=None,
                op0=OP.max,
            )
            vector.drain()
            vector.reciprocal(out=tmp[0:1, 2:3], in_=tmp[0:1, 1:2])
            vector.drain()
            vector.tensor_tensor(
                out=tmp[0:1, 3:4], in0=gsb[0:1, 0:1], in1=tmp[0:1, 2:3], op=OP.mult
            )
            vector.tensor_scalar(
                out=tmp[0:1, 4:5], in0=tmp[0:1, 0:1], scalar1=0.5, scalar2=None,
                op0=OP.is_ge,
            )
            vector.drain()
            vector.tensor_tensor(
                out=tmp[0:1, 5:6], in0=tmp[0:1, 3:4], in1=tmp[0:1, 4:5], op=OP.mult
            )
            vector.tensor_scalar(
                out=tmp[0:1, 6:7], in0=gsb[0:1, 2:3], scalar1=ALPHA / B, scalar2=None,
                op0=OP.mult,
            )
            vector.drain()
            vector.scalar_tensor_tensor(
                out=out_sb[0:1, 0:1], in0=tmp[0:1, 5:6], scalar=1.0 - ALPHA,
                in1=tmp[0:1, 6:7], op0=OP.mult, op1=OP.add,
            ).then_inc(sem_vout, 1)

        @block.scalar
        def _(scalar: bass.BassScalarEngine):
            for i in SSET:
                r = i % RT
                si = s_idx[i]
                scalar.wait_ge(sem_arg, i + 1)
                scalar.activation(
                    out=yscr[si % 2][:, :], in_=argb[i % DEPTH][:, :], func=AF.Relu,
                    bias=bias_sb[:, r : r + 1], scale=1.0,
                    accum_out=red_src[:, si : si + 1],
                ).then_inc(sem_y, 1)

        @block.tensor
        def _(tensor: bass.BassTensorEngine):
            nslice = csz // 512
            for i in range(nt):
                tensor.wait_ge(sem_pen, i + 1)
                for s in range(nslice):
                    mm = tensor.matmul(
                        out=psum_h[0:1, :],
                        lhsT=ones_bf[:, 0:1],
                        rhs=pen[i % DEPTH][:, s * 512 : (s + 1) * 512],
                        start=(i == 0 and s == 0),
                        stop=(i == nt - 1 and s == nslice - 1),
                    )
                    if s == nslice - 1:
                        mm.then_inc(sem_tt, 1)
                if i in v_idx:
                    vi = v_idx[i]
                    tensor.wait_ge(sem_yv, vi + 1)
                    for s in range(nslice):
                        mm = tensor.matmul(
                            out=psum_y[0:1, :],
                            lhsT=ones_bf[:, 0:1],
                            rhs=yv[vi % 2][:, s * 512 : (s + 1) * 512],
                            start=(vi == 0 and s == 0),
                            stop=(vi == nv - 1 and s == nslice - 1),
                        )
                        if s == nslice - 1:
                            mm.then_inc(sem_ty, 1)
            # final reduction of ScalarE sums + mse
            tensor.wait_ge(sem_vdone, 1)
            tensor.wait_ge(sem_y, ns)
            tensor.matmul(
                out=psum_red[0:1, :],
                lhsT=ones_sb[:, 0:1],
                rhs=red_src[:, :],
                start=True,
                stop=True,
            ).then_inc(sem_t, 1)

    return nc


_NC_CACHE = {}


def _get_nc(mode=None):
    mode = mode or MODE
    if mode not in _NC_CACHE:
        _NC_CACHE[mode] = build(mode)
    return _NC_CACHE[mode]


def make_in_maps(predictions: np.ndarray, targets: np.ndarray):
    p = np.ascontiguousarray(predictions.reshape(B).astype(np.float32))
    t = np.ascontiguousarray(targets.reshape(B).astype(np.float32))
    in_maps = []
    for c in range(NCORES):
        sl = slice(c * ROWS_PER_CORE, (c + 1) * ROWS_PER_CORE)
        in_maps.append(
            {
                "t_rows": t[sl].reshape(P, RT).copy(),
                "p_rows": p[sl].reshape(P, RT).copy(),
                "t_full": t.reshape(1, B).copy(),
                "p_full": p.reshape(1, B).copy(),
            }
        )
    return in_maps


def kernel(predictions: np.ndarray, targets: np.ndarray) -> np.ndarray:
    nc = _get_nc()
    in_maps = make_in_maps(predictions, targets)
    res = run_bass_kernel_spmd(nc, in_maps, core_ids=list(range(NCORES)))
    out = res.results[0]["out"]
    return np.float32(out.reshape(())[()])


# revision 21
# speedup vs baseline: 1.1170x; 1.0722x over previous
"""AlphaRankingLoss on 8 TRN2 NeuronCores (Bass SPMD kernel).

reference math:
    mse  = mean((p - t)^2)
    pair_loss[i,j] = relu(0.5 - sign(t_i - t_j) * (p_i - p_j)),  i<j, t_i != t_j
    loss = 0.7 * mse + 0.3 * sum(pair_loss)/count

The pair term is symmetric under (i,j) swap, so the triu sum over non-tied
pairs equals the sum over ordered pairs with t_i > t_j of
relu(0.5 - (p_i - p_j)); the count equals #{(i,j): t_i > t_j}.  Each core
computes 1024 rows x 8192 cols of that ordered-pair matrix, tiled as
[128 rows x CHUNK cols]:

  VectorE: pen[i,j] = -BIG * (t_j >= t_i)        tensor_scalar is_ge+mult (4x)
           arg[i,j] = pen[i,j] + p_j             tensor_tensor add (2x)
  ScalarE: y = relu(arg + (0.5 - p_i)), fused row-sum accumulate
           (a subset of tiles does this on VectorE instead, with the sums
            taken by TensorE, to balance the two engines)
  TensorE: ones^T @ pen tiles accumulated into PSUM -> -BIG * pair count
           ones^T @ y tiles (VectorE-relu'd subset) -> their pair sums

The per-core partials (pair sum, -BIG*count, mse sum) are AllReduce'd
across the 8 cores and the final scalar math runs on VectorE.
"""

import numpy as np

import concourse.bass as bass
import concourse.mybir as mybir
from concourse.bass_utils import run_bass_kernel_spmd

# ---- problem constants (hardcoded per spec) ----
B = 8192
NCORES = 8
ROWS_PER_CORE = B // NCORES  # 1024
P = 128
RT = ROWS_PER_CORE // P  # 8 row tiles per core
ALPHA = 0.7
MARGIN = 0.5
BIG = float(2**30)

F32 = mybir.dt.float32
BF16 = mybir.dt.bfloat16

MODE = "bf16"


def build(mode=None):
    mode = mode or MODE
    if mode == "f32":
        cdt = F32
        adt = F32
        chunk = 4096
        vrelu_every = 0  # all relu on ScalarE
    else:
        cdt = BF16
        adt = BF16
        chunk = 4096
        vrelu_every = 1  # last tiles relu on VectorE
    nch = B // chunk
    nt = nch * RT
    csz = chunk
    DEPTH = 6

    if vrelu_every:
        VSET = [nt - 4, nt - 3, nt - 2, nt - 1]
    else:
        VSET = []
    SSET = [i for i in range(nt) if i not in VSET]
    ns, nv = len(SSET), len(VSET)
    s_idx = {i: si for si, i in enumerate(SSET)}
    v_idx = {i: vi for vi, i in enumerate(VSET)}

    nc = bass.Bass(num_devices=NCORES, detect_race_conditions=False)

    t_rows = nc.declare_dram_parameter("t_rows", [P, RT], F32, isOutput=False)
    p_rows = nc.declare_dram_parameter("p_rows", [P, RT], F32, isOutput=False)
    t_full = nc.declare_dram_parameter("t_full", [1, B], F32, isOutput=False)
    p_full = nc.declare_dram_parameter("p_full", [1, B], F32, isOutput=False)
    out_ext = nc.declare_dram_parameter("out", [1, 1], F32, isOutput=True)

    cc_in = nc.dram_tensor("cc_in", [1, 128], F32)
    cc_out = nc.dram_tensor("cc_out", [1, 128], F32, addr_space="Shared")
    tbf_dram = nc.dram_tensor("tbf_dram", [1, B], cdt)
    pbf_dram = nc.dram_tensor("pbf_dram", [1, B], adt)

    core_ids = list(range(NCORES))
    AX = mybir.AxisListType.X
    OP = mybir.AluOpType
    AF = mybir.ActivationFunctionType

    from contextlib import ExitStack

    ctx = ExitStack()
    with ctx:
        tb = [ctx.enter_context(nc.sbuf_tensor(f"tb{b}", [P, csz], cdt)) for b in range(2)]
        pb = [ctx.enter_context(nc.sbuf_tensor(f"pb{b}", [P, csz], adt)) for b in range(2)]
        pen = [ctx.enter_context(nc.sbuf_tensor(f"pen{b}", [P, csz], adt)) for b in range(DEPTH)]
        argb = [ctx.enter_context(nc.sbuf_tensor(f"argb{b}", [P, csz], adt)) for b in range(DEPTH)]
        yscr = [ctx.enter_context(nc.sbuf_tensor(f"yscr{b}", [P, csz], adt)) for b in range(2)]
        yv = [ctx.enter_context(nc.sbuf_tensor(f"yv{b}", [P, csz], adt)) for b in range(3)]
        t_sb = ctx.enter_context(nc.sbuf_tensor("t_sb", [P, RT], F32))
        p_sb = ctx.enter_context(nc.sbuf_tensor("p_sb", [P, RT], F32))
        tqh_sb = ctx.enter_context(nc.sbuf_tensor("tqh_sb", [P, RT], cdt))
        tq_sb = ctx.enter_context(nc.sbuf_tensor("tq_sb", [P, RT], F32))
        bias_sb = ctx.enter_context(nc.sbuf_tensor("bias_sb", [P, RT], F32))
        d_sb = ctx.enter_context(nc.sbuf_tensor("d_sb", [P, RT], F32))
        d2_sb = ctx.enter_context(nc.sbuf_tensor("d2_sb", [P, RT], F32))
        # red_src columns: [0:ns] ScalarE y sums, [ns] mse
        red_src = ctx.enter_context(nc.sbuf_tensor("red_src", [P, ns + 1], F32))
        ones_sb = ctx.enter_context(nc.sbuf_tensor("ones_sb", [P, 1], F32))
        ones_bf = ctx.enter_context(nc.sbuf_tensor("ones_bf", [P, 1], adt))
        send_sb = ctx.enter_context(nc.sbuf_tensor("send_sb", [1, 128], F32))
        gsb = ctx.enter_context(nc.sbuf_tensor("gsb", [1, 128], F32))
        tmp = ctx.enter_context(nc.sbuf_tensor("tmp", [1, 8], F32))
        out_sb = ctx.enter_context(nc.sbuf_tensor("out_sb", [1, 1], F32))
        psum_h = ctx.enter_context(nc.psum_tensor("psum_h", [1, 512], F32))
        psum_y = ctx.enter_context(nc.psum_tensor("psum_y", [1, 512], F32))
        psum_red = ctx.enter_context(nc.psum_tensor("psum_red", [1, ns + 1], F32))

        sem_ld = ctx.enter_context(nc.semaphore("sem_ld"))
        sem_pc = ctx.enter_context(nc.semaphore("sem_pc"))
        sem_bct = [ctx.enter_context(nc.semaphore(f"sem_bct{b}")) for b in range(2)]
        sem_bcp = [ctx.enter_context(nc.semaphore(f"sem_bcp{b}")) for b in range(2)]
        sem_vchunk = ctx.enter_context(nc.semaphore("sem_vchunk"))
        sem_pen = ctx.enter_context(nc.semaphore("sem_pen"))
        sem_tt = ctx.enter_context(nc.semaphore("sem_tt"))
        sem_arg = ctx.enter_context(nc.semaphore("sem_arg"))
        sem_y = ctx.enter_context(nc.semaphore("sem_y"))
        sem_yv = ctx.enter_context(nc.semaphore("sem_yv"))
        sem_ty = ctx.enter_context(nc.semaphore("sem_ty"))
        sem_vdone = ctx.enter_context(nc.semaphore("sem_vdone"))
        sem_t = ctx.enter_context(nc.semaphore("sem_t"))
        sem_vcc = ctx.enter_context(nc.semaphore("sem_vcc"))
        sem_vout = ctx.enter_context(nc.semaphore("sem_vout"))
        sem_dcc = ctx.enter_context(nc.semaphore("sem_dcc"))
        cc_sem = ctx.enter_context(nc.semaphore("cc_sem"))

        block = ctx.enter_context(nc.Block())

        cast_bcast = cdt != F32 or adt != F32

        @block.sync
        def _(sync: bass.BassEngine):
            sync.dma_start(out=t_sb[:, :], in_=t_rows[:, :]).then_inc(sem_ld, 16)
            sync.dma_start(out=p_sb[:, :], in_=p_rows[:, :]).then_inc(sem_ld, 16)
            # t-broadcasts from the bf16 scratch (gpsimd does p-broadcasts)
            sync.wait_ge(sem_pc, 32)
            for k in range(nch):
                bsem = sem_bct[k % 2]
                if k >= 2:
                    sync.wait_ge(sem_vchunk, k - 1)
                    sync.wait_ge(bsem, 16 * (k // 2))
                src_t = tbf_dram[0:1, k * csz : (k + 1) * csz].partition_broadcast(P)
                sync.dma_start(out=tb[k % 2][:, :], in_=src_t).then_inc(bsem, 16)
            sync.wait_ge(sem_vcc, 1)
            sync.dma_start(out=cc_in[0:1, :], in_=send_sb[0:1, :]).then_inc(sem_dcc, 16)
            sync.wait_ge(cc_sem, 1)
            sync.dma_start(out=gsb[0:1, 0:4], in_=cc_out[0:1, 0:4]).then_inc(sem_dcc, 16)
            sync.wait_ge(sem_vout, 1)
            sync.dma_start(out=out_ext[0:1, :], in_=out_sb[0:1, :]).then_inc(sem_dcc, 16)
            sync.wait_ge(sem_dcc, 48)

        @block.gpsimd
        def _(gpsimd: bass.BassGpSimd):
            # pre-cast f32 -> compute dtype into DRAM scratch (SWDGE casts)
            gpsimd.dma_start(out=tbf_dram[0:1, :], in_=t_full[0:1, :]).then_inc(sem_pc, 16)
            gpsimd.dma_start(out=pbf_dram[0:1, :], in_=p_full[0:1, :]).then_inc(sem_pc, 16)
            gpsimd.wait_ge(sem_pc, 32)
            for k in range(nch):
                bsem = sem_bcp[k % 2]
                if k >= 2:
                    gpsimd.wait_ge(sem_vchunk, k - 1)
                    gpsimd.wait_ge(bsem, 16 * (k // 2))
                src_p = pbf_dram[0:1, k * csz : (k + 1) * csz].partition_broadcast(P)
                gpsimd.dma_start(out=pb[k % 2][:, :], in_=src_p).then_inc(bsem, 16)
            gpsimd.wait_ge(sem_dcc, 16)
            gpsimd.collective_compute(
                "AllReduce",
                OP.add,
                replica_groups=[core_ids],
                ins=[cc_in[0:1, :]],
                outs=[cc_out[0:1, :]],
            ).then_inc(cc_sem)

        @block.vector
        def _(vector: bass.BassVectorEngine):
            vector.memset(ones_sb[:, :], 1.0)
            vector.memset(ones_bf[:, :], 1.0)
            vector.memset(send_sb[0:1, :], 0.0)
            vector.wait_ge(sem_ld, 32)
            # bias = MARGIN - p
            vector.tensor_scalar(
                out=bias_sb[:, :], in0=p_sb[:, :],
                scalar1=-1.0, scalar2=MARGIN, op0=OP.mult, op1=OP.add,
            )
            # t rounded through the compare dtype, back to f32, so the per-row
            # scalar and the broadcast tensor quantize identically
            vector.tensor_copy(out=tqh_sb[:, :], in_=t_sb[:, :])
            # mse partial: sum((p - t)^2) over this core's rows
            vector.tensor_tensor(
                out=d_sb[:, :], in0=p_sb[:, :], in1=t_sb[:, :], op=OP.subtract
            )
            vector.drain()
            vector.tensor_copy(out=tq_sb[:, :], in_=tqh_sb[:, :])
            vector.scalar_tensor_tensor(
                out=d2_sb[:, :], in0=d_sb[:, :], scalar=0.0, in1=d_sb[:, :],
                op0=OP.add, op1=OP.mult,
                accum_out=red_src[:, ns : ns + 1],
            )
            vector.drain()
            i = 0
            for k in range(nch):
                vector.wait_ge(sem_bct[k % 2], 16 * (k // 2 + 1))
                vector.wait_ge(sem_bcp[k % 2], 16 * (k // 2 + 1))
                for r in range(RT):
                    # pen = -BIG * (t_j >= t_i)
                    if i >= DEPTH:
                        vector.wait_ge(sem_tt, i - DEPTH + 1)
                    vector.tensor_scalar(
                        out=pen[i % DEPTH][:, :], in0=tb[k % 2][:, :],
                        scalar1=tq_sb[:, r : r + 1], scalar2=-BIG,
                        op0=OP.is_ge, op1=OP.mult,
                    ).then_inc(sem_pen, 1)
                    # arg = pen + p_j
                    if i >= DEPTH and (i - DEPTH) in s_idx:
                        vector.wait_ge(sem_y, s_idx[i - DEPTH] + 1)
                    vector.tensor_tensor(
                        out=argb[i % DEPTH][:, :], in0=pen[i % DEPTH][:, :],
                        in1=pb[k % 2][:, :], op=OP.add,
                    ).then_inc(sem_arg, 1)
                    if i in v_idx:
                        vi = v_idx[i]
                        if vi >= 3:
                            vector.wait_ge(sem_ty, vi - 2)
                        # y = relu(arg + bias)
                        vector.tensor_scalar(
                            out=yv[vi % 3][:, :], in0=argb[i % DEPTH][:, :],
                            scalar1=bias_sb[:, r : r + 1], scalar2=0.0,
                            op0=OP.add, op1=OP.max,
                        ).then_inc(sem_yv, 1)
                    i += 1
                vector.memset(tmp[0:1, 7:8], 0.0).then_inc(sem_vchunk, 1)
            vector.memset(tmp[0:1, 6:7], 0.0).then_inc(sem_vdone, 1)

            # ---- final reduction ----
            vector.wait_ge(sem_t, 1)
            vector.wait_ge(sem_tt, nt)
            if nv:
                vector.wait_ge(sem_ty, nv)
            # pair sum = sum(ScalarE col sums) + sum(psum_y)
            vector.tensor_reduce(
                out=tmp[0:1, 0:1], in_=psum_red[0:1, 0:ns], axis=AX, op=OP.add
            )
            if nv:
                vector.tensor_reduce(
                    out=tmp[0:1, 1:2], in_=psum_y[0:1, :], axis=AX, op=OP.add
                )
            else:
                vector.memset(tmp[0:1, 1:2], 0.0)
            vector.tensor_reduce(
                out=send_sb[0:1, 1:2], in_=psum_h[0:1, :], axis=AX, op=OP.add
            )
            vector.tensor_copy(out=send_sb[0:1, 2:3], in_=psum_red[0:1, ns : ns + 1])
            vector.drain()
            vector.tensor_tensor(
                out=send_sb[0:1, 0:1], in0=tmp[0:1, 0:1], in1=tmp[0:1, 1:2], op=OP.add
            )
            vector.drain()
            vector.memset(tmp[0:1, 5:6], 0.0).then_inc(sem_vcc, 1)

            vector.wait_ge(sem_dcc, 32)
            # gsb[0,0]=S, gsb[0,1]=-BIG*count (+noise), gsb[0,2]=mse sum
            # C = B^2 + gsb[0,1]/BIG
            vector.tensor_scalar(
                out=tmp[0:1, 0:1], in0=gsb[0:1, 1:2],
                scalar1=1.0 / BIG, scalar2=float(B * B), op0=OP.mult, op1=OP.add,
            )
            vector.drain()
            vector.tensor_scalar(
                out=tmp[0:1, 1:2], in0=tmp[0:1, 0:1], scalar1=1.0, scalar2=None,
                op0=OP.max,
            )
            vector.drain()
            vector.reciprocal(out=tmp[0:1, 2:3], in_=tmp[0:1, 1:2])
            vector.drain()
            vector.tensor_tensor(
                out=tmp[0:1, 3:4], in0=gsb[0:1, 0:1], in1=tmp[0:1, 2:3], op=OP.mult
            )
            vector.tensor_scalar(
                out=tmp[0:1, 4:5], in0=tmp[0:1, 0:1], scalar1=0.5, scalar2=None,
                op0=OP.is_ge,
            )
            vector.drain()
            vector.tensor_tensor(
                out=tmp[0:1, 5:6], in0=tmp[0:1, 3:4], in1=tmp[0:1, 4:5], op=OP.mult
            )
            vector.tensor_scalar(
                out=tmp[0:1, 6:7], in0=gsb[0:1, 2:3], scalar1=ALPHA / B, scalar2=None,
                op0=OP.mult,
            )
            vector.drain()
            vector.scalar_tensor_tensor(
                out=out_sb[0:1, 0:1], in0=tmp[0:1, 5:6], scalar=1.0 - ALPHA,
                in1=tmp[0:1, 6:7], op0=OP.mult, op1=OP.add,
            ).then_inc(sem_vout, 1)

        @block.scalar
        def _(scalar: bass.BassScalarEngine):
            for i in SSET:
                r = i % RT
                si = s_idx[i]
                scalar.wait_ge(sem_arg, i + 1)
                scalar.activation(
                    out=yscr[si % 2][:, :], in_=argb[i % DEPTH][:, :], func=AF.Relu,
                    bias=bias_sb[:, r : r + 1], scale=1.0,
                    accum_out=red_src[:, si : si + 1],
                ).then_inc(sem_y, 1)

        @block.tensor
        def _(tensor: bass.BassTensorEngine):
            nslice = csz // 512
            for i in range(nt):
                tensor.wait_ge(sem_pen, i + 1)
                for s in range(nslice):
                    mm = tensor.matmul(
                        out=psum_h[0:1, :],
                        lhsT=ones_bf[:, 0:1],
                        rhs=pen[i % DEPTH][:, s * 512 : (s + 1) * 512],
                        start=(i == 0 and s == 0),
                        stop=(i == nt - 1 and s == nslice - 1),
                    )
                    if s == nslice - 1:
                        mm.then_inc(sem_tt, 1)
                if i in v_idx:
                    vi = v_idx[i]
                    tensor.wait_ge(sem_yv, vi + 1)
                    for s in range(nslice):
                        mm = tensor.matmul(
                            out=psum_y[0:1, :],
                            lhsT=ones_bf[:, 0:1],
                            rhs=yv[vi % 3][:, s * 512 : (s + 1) * 512],
                            start=(vi == 0 and s == 0),
                            stop=(vi == nv - 1 and s == nslice - 1),
                        )
                        if s == nslice - 1:
                            mm.then_inc(sem_ty, 1)
            # final reduction of ScalarE sums + mse
            tensor.wait_ge(sem_vdone, 1)
            tensor.wait_ge(sem_y, ns)
            tensor.matmul(
                out=psum_red[0:1, :],
                lhsT=ones_sb[:, 0:1],
                rhs=red_src[:, :],
                start=True,
                stop=True,
            ).then_inc(sem_t, 1)

    return nc


_NC_CACHE = {}


def _get_nc(mode=None):
    mode = mode or MODE
    if mode not in _NC_CACHE:
        _NC_CACHE[mode] = build(mode)
    return _NC_CACHE[mode]


def make_in_maps(predictions: np.ndarray, targets: np.ndarray):
    p = np.ascontiguousarray(predictions.reshape(B).astype(np.float32))
    t = np.ascontiguousarray(targets.reshape(B).astype(np.float32))
    in_maps = []
    for c in range(NCORES):
        sl = slice(c * ROWS_PER_CORE, (c + 1) * ROWS_PER_CORE)
        in_maps.append(
            {
                "t_rows": t[sl].reshape(P, RT).copy(),
                "p_rows": p[sl].reshape(P, RT).copy(),
                "t_full": t.reshape(1, B).copy(),
                "p_full": p.reshape(1, B).copy(),
            }
        )
    return in_maps


def kernel(predictions: np.ndarray, targets: np.ndarray) -> np.ndarray:
    nc = _get_nc()
    in_maps = make_in_maps(predictions, targets)
    res = run_bass_kernel_spmd(nc, in_maps, core_ids=list(range(NCORES)))
    out = res.results[0]["out"]
    return np.float32(out.reshape(())[()])
